# revision 1
# baseline (speedup 1.0000x reference)
"""EnhancedLoRALinear Trainium2 kernel.

Computes, for x:[4,8192,1024] and torch-style weights (out,in):
    out = x @ (W + W_res)^T + b + sigmoid(x @ W_gate^T) * (2 * (x @ W_down^T) @ W_up^T)

Strategy:
  - Data-parallel: the 32768 tokens are split across 8 NeuronCores (4096 each);
    the small weight matrices are replicated.
  - Algebraic fold: main + residual share one matmul with Wc = W + W_res.
  - Host prep: weights are pre-transposed to [in, out] so the contraction dim
    lands on SBUF partitions; x shards are pre-transposed to [in, tokens] for
    the same reason. LoRA scaling (2.0, exact in fp32) is folded into W_up^T.
  - Device: fp32r matmuls (full PE rate at moving free dim >= 256). Per
    128-token tile and 512-wide output half: a K=1 ones-row matmul seeds the
    main PSUM with the bias, 8 k-tile matmuls accumulate the main path, 8 the
    gate path, one K=16 matmul applies the LoRA up-projection from an
    [R=16, 512-token] down-projection computed once per 512 tokens. Sigmoid
    runs on ScalarE; gate*lora and +main on VectorE.
  - Sync-wait budget: fp32r matmuls can encode ONE hw sync-wait, other engine
    ops TWO. Hence: each multi-part tensor loads with a single DMA instruction
    (one queue semaphore), warm-up matmuls make the PE observe every weight
    DMA semaphore first (enforced via ordering deps), and the epilogue is
    shaped so every op joins at most two semaphores.
"""

import ml_dtypes
import numpy as np

_BF16 = ml_dtypes.bfloat16

import concourse.bass as bass
import concourse.bacc as bacc
import concourse.mybir as mybir
import concourse.tile as tile
from concourse.bass_utils import run_bass_kernel_spmd
from concourse.tile_rust import add_dep_helper

N_CORES = 8
B, S = 4, 8192
TOK = B * S                  # 32768 tokens total
T = TOK // N_CORES           # 4096 tokens per core
I = 1024                     # in_features
O = 1024                     # out_features
R = 16                       # lora rank
SCALING = 2.0                # lora_alpha / r (exact power of two)
KT = I // 128                # 8 contraction tiles
TG = 512                     # token group (down-projection batch)
NG = T // TG                 # 8 groups per core
NH = O // 512                # 2 output halves

F32 = mybir.dt.float32
F32R = mybir.dt.float32r


def _build_nc():
    nc = bacc.Bacc(None)

    xt = nc.dram_tensor("xt", [I, T], F32R, kind="ExternalInput")
    wct = nc.dram_tensor("wct", [I, O], F32R, kind="ExternalInput")
    wgt = nc.dram_tensor("wgt", [I, O], mybir.dt.bfloat16, kind="ExternalInput")
    xtb = nc.dram_tensor("xtb", [I, T], mybir.dt.bfloat16, kind="ExternalInput")
    wdt = nc.dram_tensor("wdt", [I, R], F32R, kind="ExternalInput")
    wut2 = nc.dram_tensor("wut2", [R, O], F32R, kind="ExternalInput")
    biasbc = nc.dram_tensor("biasbc", [128, O], F32, kind="ExternalInput")
    out = nc.dram_tensor("out", [T, O], F32, kind="ExternalOutput")

    # [i, o] -> [p, k, o] views so each weight loads with ONE DMA instruction
    xt_v = xt.rearrange("(k p) t -> p k t", p=128)
    xtb_v = xtb.rearrange("(k p) t -> p k t", p=128)
    wct_v = wct.rearrange("(k p) o -> p k o", p=128)
    wgt_v = wgt.rearrange("(k p) o -> p k o", p=128)
    wdt_v = wdt.rearrange("(k p) r -> p k r", p=128)

    sig = mybir.ActivationFunctionType.Sigmoid
    mult = mybir.AluOpType.mult
    add = mybir.AluOpType.add

    with tile.TileContext(nc) as tc:
        with (
            tc.tile_pool(name="wpool", bufs=1) as wpool,
            tc.tile_pool(name="xpool", bufs=3) as xpool,
            tc.tile_pool(name="opool", bufs=3) as opool,
            tc.tile_pool(name="epool", bufs=3) as epool,
            tc.tile_pool(name="psum", bufs=1, space="PSUM") as pp,
        ):
            # --- resident weights, one DMA each ---
            wc_sb = wpool.tile([128, KT, O], F32R)
            wg_sb = wpool.tile([128, KT, O], mybir.dt.bfloat16)
            wd_sb = wpool.tile([128, KT, R], F32R)
            wu_sb = wpool.tile([R, O], F32R)
            bias_bc = wpool.tile([128, O], F32)

            for k in range(KT):
                nc.sync.dma_start(out=wc_sb[:, k, :], in_=wct_v[:, k, :])
            nc.sync.dma_start(out=wg_sb[:, 0:4, :], in_=wgt_v[:, 0:4, :])
            nc.sync.dma_start(out=wg_sb[:, 4:8, :], in_=wgt_v[:, 4:8, :])
            nc.sync.dma_start(out=wd_sb[:, :, :], in_=wdt_v[:, :, :])
            nc.sync.dma_start(out=wu_sb[:, :], in_=wut2[:, :])
            nc.sync.dma_start(out=bias_bc[:, :], in_=biasbc[:, :])

            # HAM spin-up: ~60 junk matmuls keep the PE busy through the DMA
            # prologue so the clock gate opens before real compute starts
            junk = wpool.tile([128, 512], mybir.dt.bfloat16)
            nc.gpsimd.memset(junk[:, :], 0.0)
            warm = pp.tile([128, 512], F32, tag="warm")
            spin = None
            for i in range(110):
                spin = nc.tensor.matmul(warm[:, :], junk[:, 0:128], junk[:, :],
                                        start=True, stop=True)

            # warm-up matmuls: make the PE observe each weight-DMA semaphore
            # (fp32r matmuls can encode only one sync-wait downstream)
            warms = [
                nc.tensor.matmul(warm[0:1, :], wc_sb[:, k, 0:1],
                                 wc_sb[:, k, 0:512], start=True, stop=True)
                for k in range(KT)
            ] + [
                nc.tensor.matmul(warm[0:1, :], wg_sb[:, 0, 0:1],
                                 wg_sb[:, 0, 0:512], start=True, stop=True),
                nc.tensor.matmul(warm[0:1, :], wg_sb[:, 4, 0:1],
                                 wg_sb[:, 4, 0:512], start=True, stop=True),
                nc.tensor.matmul(warm[0:16, 0:16], wd_sb[:, 0, :],
                                 wd_sb[:, 0, :], start=True, stop=True),
                nc.tensor.matmul(warm[0:1, :], wu_sb[:, 0:1],
                                 wu_sb[:, 0:512], start=True, stop=True),
                spin,
            ]
            first_real = []  # first matmul of each psum group in group 0

            for g in range(NG):
                tg0 = g * TG
                xt_sb = xpool.tile([128, KT, TG], F32R, tag="xt")
                nc.sync.dma_start(
                    out=xt_sb[:, :, :], in_=xt_v[:, :, tg0 : tg0 + TG]
                )
                xtb_sb = xpool.tile([128, KT, TG], mybir.dt.bfloat16, tag="xtb")
                nc.sync.dma_start(
                    out=xtb_sb[:, :, :], in_=xtb_v[:, :, tg0 : tg0 + TG]
                )

                # LoRA down-projection for the whole 512-token group: [R, TG]
                dps = pp.tile([R, TG], F32, tag="misc")
                for k in range(KT):
                    mm = nc.tensor.matmul(
                        dps[:, :],
                        wd_sb[:, k, :],
                        xt_sb[:, k, :],
                        start=(k == 0),
                        stop=(k == KT - 1),
                    )
                    if g == 0 and k == 0:
                        first_real.append(mm)
                down_sb = epool.tile([R, TG], F32R, tag="down")
                nc.vector.tensor_copy(down_sb[:, :], dps[:, :])

                for t in range(TG // 128):
                    tsl = slice(t * 128, (t + 1) * 128)
                    out_sb = opool.tile([128, O], F32, tag="out")
                    for oh in range(NH):
                        osl = slice(oh * 512, (oh + 1) * 512)
                        mps = pp.tile([128, 512], F32, tag=f"main{oh}")
                        gps = pp.tile([128, 512], F32, tag=f"gate{oh}")
                        lps = pp.tile([128, 512], F32, tag=f"lora{oh}")
                        for k in range(KT):
                            mm = nc.tensor.matmul(
                                mps[:, :],
                                xt_sb[:, k, tsl],
                                wc_sb[:, k, osl],
                                start=(k == 0),
                                stop=(k == KT - 1),
                            )
                            if g == 0 and t == 0 and k == 0:
                                first_real.append(mm)
                        for k in range(KT):
                            nc.tensor.ldweights(xtb_sb[:, k, tsl])
                            mm = nc.tensor.matmul(
                                gps[:, :],
                                xtb_sb[:, k, tsl],
                                wg_sb[:, k, osl],
                                start=(k == 0),
                                stop=(k == KT - 1),
                            )
                            if g == 0 and t == 0 and k == 0:
                                first_real.append(mm)
                        mm = nc.tensor.matmul(
                            lps[:, :],
                            down_sb[:, tsl],
                            wu_sb[:, osl],
                            start=True,
                            stop=True,
                        )
                        if g == 0 and t == 0:
                            first_real.append(mm)
                        g_sb = epool.tile([128, 512], F32, tag="sig")
                        nc.scalar.activation(g_sb[:, :], gps[:, :], sig)
                        gl_sb = epool.tile([128, 512], F32, tag="gl")
                        nc.vector.tensor_tensor(
                            gl_sb[:, :], g_sb[:, :], lps[:, :], mult
                        )
                        nc.gpsimd.tensor_tensor(
                            gl_sb[:, :], gl_sb[:, :], bias_bc[:, osl], add
                        )
                        nc.vector.tensor_tensor(
                            out_sb[:, osl], gl_sb[:, :], mps[:, :], add
                        )
                    nc.sync.dma_start(
                        out=out[tg0 + t * 128 : tg0 + (t + 1) * 128, :],
                        in_=out_sb[:, :],
                    )

            # ordering-only deps: every warm-up precedes the first matmul of
            # each group-0 psum chain, so no real matmul lands before the PE
            # has observed all weight DMA semaphores
            for w in warms:
                for fr in first_real:
                    add_dep_helper(fr.ins, w.ins, False,
                                   "warmups before real matmuls")
    nc.compile()
    return nc


_NC_CACHE = None


def _get_nc():
    global _NC_CACHE
    if _NC_CACHE is None:
        _NC_CACHE = _build_nc()
    return _NC_CACHE


def _prep_inputs(x, W, b, W_down, W_up, W_gate, W_res):
    x = np.asarray(x, dtype=np.float32).reshape(TOK, I)
    wct = np.ascontiguousarray((np.asarray(W) + np.asarray(W_res)).T.astype(np.float32))
    wgt = np.ascontiguousarray(np.asarray(W_gate).T.astype(_BF16))
    wdt = np.ascontiguousarray(np.asarray(W_down).T.astype(np.float32))
    wut2 = np.ascontiguousarray((SCALING * np.asarray(W_up)).T.astype(np.float32))
    biasbc = np.ascontiguousarray(
        np.broadcast_to(np.asarray(b, dtype=np.float32).reshape(1, O), (128, O))
    )
    in_maps = []
    for c in range(N_CORES):
        xt_c = np.ascontiguousarray(x[c * T : (c + 1) * T, :].T)
        xtb_c = xt_c.astype(_BF16)
        in_maps.append(
            {
                "xt": xt_c,
                "xtb": xtb_c,
                "wct": wct,
                "wgt": wgt,
                "wdt": wdt,
                "wut2": wut2,
                "biasbc": biasbc,
            }
        )
    return in_maps


def run(inputs, trace=False, **kwargs):
    """Build + run on the 8 NeuronCores. Returns (full_output, BassKernelResults)."""
    nc = _get_nc()
    in_maps = _prep_inputs(**inputs)
    res = run_bass_kernel_spmd(
        nc, in_maps, list(range(N_CORES)), trace=trace, **kwargs
    )
    shards = [res.results[c]["out"] for c in range(N_CORES)]
    full = np.concatenate(shards, axis=0).reshape(B, S, O)
    return full, res


def kernel(**inputs):
    out, _ = run(inputs, trace=False)
    return out



# revision 3
# speedup vs baseline: 1.4053x; 1.4053x over previous
"""EnhancedLoRALinear Trainium2 kernel.

Computes, for x:[4,8192,1024] and torch-style weights (out,in):
    out = x @ (W + W_res)^T + b + sigmoid(x @ W_gate^T) * (2 * (x @ W_down^T) @ W_up^T)

Strategy (v2 — fp8 DoubleRow gate/down, bf16 main):
  - Data-parallel: the 32768 tokens are split across 8 NeuronCores (4096 each);
    the small weight matrices are replicated.
  - Algebraic fold: main + residual share one matmul with Wc = W + W_res.
  - Dtypes by accuracy need (tolerance 2e-2, measured host-side ~5e-3):
      main path  : bf16 x / bf16 Wc           (full-rate MM, FWL on LDWEIGHTS)
      gate path  : fp8e4 x / fp8e4 64*W_gate  (DoubleRow: 2 k-tiles per MM,
                   the x64 weight scale keeps values out of fp8 subnormals;
                   undone for free via sigmoid's scale=1/64)
      down path  : fp8e4 DoubleRow, 64*W_down; up path bf16 with (2/64)*W_up
  - Per 128-token tile and 512-wide output half: 8 bf16 main matmuls, 4 fp8
    DoubleRow gate matmuls, one K=16 up-projection from an [R=16, 512-token]
    down-projection computed once per 512 tokens (4 DoubleRow matmuls).
  - Epilogue rebalanced so each PSUM bank is drained as early as possible:
    m2 = mps + bias on GpSimd (concurrent with ScalarE sigmoid), then
    gl = g*lps and out = m2 + gl on VectorE.
  - Sync-wait budget: matmuls can encode ONE hw sync-wait, other engine ops
    TWO. Each multi-part tensor loads with few DMA instructions, warm-up
    matmuls make the PE observe every weight DMA semaphore first (enforced
    via ordering deps), junk matmuls keep HAM warm through the DMA prologue.
"""

import ml_dtypes
import numpy as np

_BF16 = ml_dtypes.bfloat16
_F8 = ml_dtypes.float8_e4m3

import concourse.bass as bass
import concourse.bacc as bacc
import concourse.mybir as mybir
import concourse.tile as tile
from concourse.bass_utils import run_bass_kernel_spmd
from concourse.tile_rust import add_dep_helper

N_CORES = 8
B, S = 4, 8192
TOK = B * S                  # 32768 tokens total
T = TOK // N_CORES           # 4096 tokens per core
I = 1024                     # in_features
O = 1024                     # out_features
R = 16                       # lora rank
KT = I // 128                # 8 contraction tiles
TG = 512                     # token group (down-projection batch)
NG = T // TG                 # 8 groups per core
NH = O // 512                # 2 output halves
WS = 64.0                    # fp8 weight pre-scale (power of two, exact)

F32 = mybir.dt.float32
BF16 = mybir.dt.bfloat16
F8E4 = mybir.dt.float8e4
DR = mybir.MatmulPerfMode.DoubleRow


def _build_nc():
    nc = bacc.Bacc(None)

    xb = nc.dram_tensor("xb", [I, T], BF16, kind="ExternalInput")
    x8 = nc.dram_tensor("x8", [I, T], F8E4, kind="ExternalInput")
    wcb = nc.dram_tensor("wcb", [I, O], BF16, kind="ExternalInput")
    wg8 = nc.dram_tensor("wg8", [I, O], F8E4, kind="ExternalInput")
    wd8 = nc.dram_tensor("wd8", [I, R], F8E4, kind="ExternalInput")
    wub = nc.dram_tensor("wub", [R, O], BF16, kind="ExternalInput")
    biasbc = nc.dram_tensor("biasbc", [128, O], F32, kind="ExternalInput")
    out = nc.dram_tensor("out", [T, O], F32, kind="ExternalOutput")

    # [i, *] -> [p, k, *] views so tensors load with few DMA instructions
    xb_v = xb.rearrange("(k p) t -> p k t", p=128)
    x8_v = x8.rearrange("(k p) t -> p k t", p=128)
    wcb_v = wcb.rearrange("(k p) o -> p k o", p=128)
    wg8_v = wg8.rearrange("(k p) o -> p k o", p=128)
    wd8_v = wd8.rearrange("(k p) r -> p k r", p=128)

    sig = mybir.ActivationFunctionType.Sigmoid
    mult = mybir.AluOpType.mult
    add = mybir.AluOpType.add

    with tile.TileContext(nc) as tc:
        with (
            tc.tile_pool(name="wpool", bufs=1) as wpool,
            tc.tile_pool(name="xpool", bufs=3) as xpool,
            tc.tile_pool(name="opool", bufs=3) as opool,
            tc.tile_pool(name="epool", bufs=3) as epool,
            tc.tile_pool(name="psum", bufs=1, space="PSUM") as pp,
        ):
            # --- resident weights ---
            wc_sb = wpool.tile([128, KT, O], BF16)
            wg_sb = wpool.tile([128, KT, O], F8E4)
            wd_sb = wpool.tile([128, KT, R], F8E4)
            wu_sb = wpool.tile([R, O], BF16)
            bias_bc = wpool.tile([128, O], F32)

            for kk in range(KT // 2):
                nc.sync.dma_start(
                    out=wc_sb[:, 2 * kk : 2 * kk + 2, :],
                    in_=wcb_v[:, 2 * kk : 2 * kk + 2, :],
                )
            nc.sync.dma_start(out=wg_sb[:, 0:4, :], in_=wg8_v[:, 0:4, :])
            nc.sync.dma_start(out=wg_sb[:, 4:8, :], in_=wg8_v[:, 4:8, :])
            nc.sync.dma_start(out=wd_sb[:, :, :], in_=wd8_v[:, :, :])
            nc.sync.dma_start(out=wu_sb[:, :], in_=wub[:, :])
            nc.sync.dma_start(out=bias_bc[:, :], in_=biasbc[:, :])

            # HAM spin-up: junk matmuls keep the PE busy through the DMA
            # prologue so the clock gate opens before real compute starts
            junk = wpool.tile([128, 512], BF16)
            nc.gpsimd.memset(junk[:, :], 0.0)
            warm = pp.tile([128, 512], F32, tag="warm")
            spin = None
            for i in range(70):
                spin = nc.tensor.matmul(warm[:, :], junk[:, 0:128], junk[:, :],
                                        start=True, stop=True)

            # warm-up matmuls: make the PE observe each weight-DMA semaphore
            # (matmuls can encode only one sync-wait downstream)
            warms = [
                nc.tensor.matmul(warm[0:1, :], wc_sb[:, 2 * kk, 0:1],
                                 wc_sb[:, 2 * kk, 0:512], start=True, stop=True)
                for kk in range(KT // 2)
            ] + [
                nc.tensor.matmul(warm[0:1, :], wg_sb[:, 0, 0:1],
                                 wg_sb[:, 0, 0:512], start=True, stop=True),
                nc.tensor.matmul(warm[0:1, :], wg_sb[:, 4, 0:1],
                                 wg_sb[:, 4, 0:512], start=True, stop=True),
                nc.tensor.matmul(warm[0:16, 0:16], wd_sb[:, 0, :],
                                 wd_sb[:, 0, :], start=True, stop=True),
                nc.tensor.matmul(warm[0:1, :], wu_sb[:, 0:1],
                                 wu_sb[:, 0:512], start=True, stop=True),
                spin,
            ]
            first_real = []  # first matmul of each psum group in group 0

            for g in range(NG):
                tg0 = g * TG
                xb_sb = xpool.tile([128, KT, TG], BF16, tag="xb")
                nc.sync.dma_start(
                    out=xb_sb[:, :, :], in_=xb_v[:, :, tg0 : tg0 + TG]
                )
                x8_sb = xpool.tile([128, KT, TG], F8E4, tag="x8")
                nc.sync.dma_start(
                    out=x8_sb[:, :, :], in_=x8_v[:, :, tg0 : tg0 + TG]
                )

                # LoRA down-projection for the whole 512-token group: [R, TG]
                # (values are 64*down because of the wd8 pre-scale)
                dps = pp.tile([R, TG], F32, tag="misc")
                for kk in range(KT // 2):
                    mm = nc.tensor.matmul(
                        dps[:, :],
                        wd_sb[:, 2 * kk : 2 * kk + 2, :],
                        x8_sb[:, 2 * kk : 2 * kk + 2, :],
                        start=(kk == 0),
                        stop=(kk == KT // 2 - 1),
                        perf_mode=DR,
                    )
                    if g == 0 and kk == 0:
                        first_real.append(mm)
                down_sb = epool.tile([R, TG], BF16, tag="down")
                nc.vector.tensor_copy(down_sb[:, :], dps[:, :])

                for t in range(TG // 128):
                    tsl = slice(t * 128, (t + 1) * 128)
                    out_sb = opool.tile([128, O], F32, tag="out")
                    for oh in range(NH):
                        osl = slice(oh * 512, (oh + 1) * 512)
                        mps = pp.tile([128, 512], F32, tag=f"main{oh}")
                        gps = pp.tile([128, 512], F32, tag=f"gate{oh}")
                        lps = pp.tile([128, 512], F32, tag=f"lora{oh}")
                        for k in range(KT):
                            mm = nc.tensor.matmul(
                                mps[:, :],
                                xb_sb[:, k, tsl],
                                wc_sb[:, k, osl],
                                start=(k == 0),
                                stop=(k == KT - 1),
                            )
                            if g == 0 and t == 0 and k == 0:
                                first_real.append(mm)
                        for kk in range(KT // 2):
                            mm = nc.tensor.matmul(
                                gps[:, :],
                                x8_sb[:, 2 * kk : 2 * kk + 2, tsl],
                                wg_sb[:, 2 * kk : 2 * kk + 2, osl],
                                start=(kk == 0),
                                stop=(kk == KT // 2 - 1),
                                perf_mode=DR,
                            )
                            if g == 0 and t == 0 and kk == 0:
                                first_real.append(mm)
                        mm = nc.tensor.matmul(
                            lps[:, :],
                            down_sb[:, tsl],
                            wu_sb[:, osl],
                            start=True,
                            stop=True,
                        )
                        if g == 0 and t == 0:
                            first_real.append(mm)
                        # epilogue: VectorE drains mps early (GpSimd cannot
                        # read PSUM) while ScalarE computes the sigmoid
                        # (whose scale undoes the fp8 x64 weight pre-scale);
                        # the final SBUF-only add goes to GpSimd
                        m2_sb = epool.tile([128, 512], F32, tag="m2")
                        nc.vector.tensor_tensor(
                            m2_sb[:, :], mps[:, :], bias_bc[:, osl], add
                        )
                        g_sb = epool.tile([128, 512], F32, tag="sig")
                        nc.scalar.activation(
                            g_sb[:, :], gps[:, :], sig, scale=1.0 / WS
                        )
                        gl_sb = epool.tile([128, 512], F32, tag="gl")
                        nc.vector.tensor_tensor(
                            gl_sb[:, :], g_sb[:, :], lps[:, :], mult
                        )
                        nc.gpsimd.tensor_tensor(
                            out_sb[:, osl], gl_sb[:, :], m2_sb[:, :], add
                        )
                    nc.sync.dma_start(
                        out=out[tg0 + t * 128 : tg0 + (t + 1) * 128, :],
                        in_=out_sb[:, :],
                    )

            # ordering-only deps: every warm-up precedes the first matmul of
            # each group-0 psum chain, so no real matmul lands before the PE
            # has observed all weight DMA semaphores
            for w in warms:
                for fr in first_real:
                    add_dep_helper(fr.ins, w.ins, False,
                                   "warmups before real matmuls")
    nc.compile()
    return nc


_NC_CACHE = None


def _get_nc():
    global _NC_CACHE
    if _NC_CACHE is None:
        _NC_CACHE = _build_nc()
    return _NC_CACHE


def _prep_inputs(x, W, b, W_down, W_up, W_gate, W_res):
    x = np.asarray(x, dtype=np.float32).reshape(TOK, I)
    wcb = np.ascontiguousarray(
        (np.asarray(W) + np.asarray(W_res)).T.astype(_BF16)
    )
    wg8 = np.ascontiguousarray((WS * np.asarray(W_gate)).T.astype(_F8))
    wd8 = np.ascontiguousarray((WS * np.asarray(W_down)).T.astype(_F8))
    wub = np.ascontiguousarray(((2.0 / WS) * np.asarray(W_up)).T.astype(_BF16))
    biasbc = np.ascontiguousarray(
        np.broadcast_to(np.asarray(b, dtype=np.float32).reshape(1, O), (128, O))
    )
    in_maps = []
    for c in range(N_CORES):
        xt_c = np.ascontiguousarray(x[c * T : (c + 1) * T, :].T)
        in_maps.append(
            {
                "xb": xt_c.astype(_BF16),
                "x8": xt_c.astype(_F8),
                "wcb": wcb,
                "wg8": wg8,
                "wd8": wd8,
                "wub": wub,
                "biasbc": biasbc,
            }
        )
    return in_maps


def run(inputs, trace=False, **kwargs):
    """Build + run on the 8 NeuronCores. Returns (full_output, BassKernelResults)."""
    nc = _get_nc()
    in_maps = _prep_inputs(**inputs)
    res = run_bass_kernel_spmd(
        nc, in_maps, list(range(N_CORES)), trace=trace, **kwargs
    )
    shards = [res.results[c]["out"] for c in range(N_CORES)]
    full = np.concatenate(shards, axis=0).reshape(B, S, O)
    return full, res


def kernel(**inputs):
    out, _ = run(inputs, trace=False)
    return out


# revision 5
# speedup vs baseline: 1.4617x; 1.0401x over previous
"""EnhancedLoRALinear Trainium2 kernel.

Computes, for x:[4,8192,1024] and torch-style weights (out,in):
    out = x @ (W + W_res)^T + b + sigmoid(x @ W_gate^T) * (2 * (x @ W_down^T) @ W_up^T)

Strategy (v3 — fp8 DoubleRow gate/down, bf16 main, pipelined prologue):
  - Data-parallel: the 32768 tokens are split across 8 NeuronCores (4096 each);
    the small weight matrices are replicated.
  - Algebraic fold: main + residual share one matmul with Wc = W + W_res.
  - Dtypes by accuracy need (tolerance 2e-2, measured ~4e-3):
      main path  : bf16 x / bf16 Wc           (full-rate MM, FWL on LDWEIGHTS)
      gate path  : fp8e4 x / fp8e4 64*W_gate  (DoubleRow: 2 k-tiles per MM,
                   the x64 weight scale keeps values out of fp8 subnormals;
                   undone for free via sigmoid's scale=1/64)
      down path  : fp8e4 DoubleRow, 64*W_down; up path bf16 with (2/64)*W_up
  - Prologue: DMAs are issued in consumption order (wd8, x8/xb of group 0,
    Wc k-pairs, W_up/bias, W_gate halves) so the PE starts real matmuls as
    soon as the first operands land instead of waiting for all weights.
    A short junk-matmul spin covers the engine-startup window for HAM.
  - Matmuls per 128-token tile are interleaved over the two 512-wide output
    halves (k-outer), so each stationary-operand load has a two-matmul
    window to hide in.
  - Sync-wait budget: matmuls can encode ONE hw sync-wait. DMA issue order
    guarantees most chains need at most one unobserved DMA semaphore; the
    two spots that would need two (down: wd8+x8, main t0: xb+wcb) get a
    warm-up matmul that observes the weight semaphore first (ordering deps).
  - Epilogue: VectorE drains mps early (m2 = mps + bias; GpSimd cannot read
    PSUM), ScalarE does the sigmoid, VectorE the gate*lora, GpSimd the final
    SBUF-only add.
"""

import ml_dtypes
import numpy as np

_BF16 = ml_dtypes.bfloat16
_F8 = ml_dtypes.float8_e4m3

import concourse.bass as bass
import concourse.bacc as bacc
import concourse.mybir as mybir
import concourse.tile as tile
from concourse.bass_utils import run_bass_kernel_spmd
from concourse.tile_rust import add_dep_helper

N_CORES = 8
B, S = 4, 8192
TOK = B * S                  # 32768 tokens total
T = TOK // N_CORES           # 4096 tokens per core
I = 1024                     # in_features
O = 1024                     # out_features
R = 16                       # lora rank
KT = I // 128                # 8 contraction tiles
TG = 512                     # token group (down-projection batch)
NG = T // TG                 # 8 groups per core
NH = O // 512                # 2 output halves
WS = 64.0                    # fp8 weight pre-scale (power of two, exact)

F32 = mybir.dt.float32
BF16 = mybir.dt.bfloat16
F8E4 = mybir.dt.float8e4
DR = mybir.MatmulPerfMode.DoubleRow


def _build_nc():
    nc = bacc.Bacc(None)

    xb = nc.dram_tensor("xb", [I, T], BF16, kind="ExternalInput")
    x8 = nc.dram_tensor("x8", [I, T], F8E4, kind="ExternalInput")
    wcb = nc.dram_tensor("wcb", [I, O], BF16, kind="ExternalInput")
    wg8 = nc.dram_tensor("wg8", [I, O], F8E4, kind="ExternalInput")
    wd8 = nc.dram_tensor("wd8", [I, R], F8E4, kind="ExternalInput")
    wub = nc.dram_tensor("wub", [R, O], BF16, kind="ExternalInput")
    biasbc = nc.dram_tensor("biasbc", [128, O], F32, kind="ExternalInput")
    out = nc.dram_tensor("out", [T, O], F32, kind="ExternalOutput")

    # [i, *] -> [p, k, *] views so tensors load with few DMA instructions
    xb_v = xb.rearrange("(k p) t -> p k t", p=128)
    x8_v = x8.rearrange("(k p) t -> p k t", p=128)
    wcb_v = wcb.rearrange("(k p) o -> p k o", p=128)
    wg8_v = wg8.rearrange("(k p) o -> p k o", p=128)
    wd8_v = wd8.rearrange("(k p) r -> p k r", p=128)

    sig = mybir.ActivationFunctionType.Sigmoid
    mult = mybir.AluOpType.mult
    add = mybir.AluOpType.add

    with tile.TileContext(nc) as tc:
        with (
            tc.tile_pool(name="wpool", bufs=1) as wpool,
            tc.tile_pool(name="xpool", bufs=3) as xpool,
            tc.tile_pool(name="opool", bufs=3) as opool,
            tc.tile_pool(name="epool", bufs=3) as epool,
            tc.tile_pool(name="psum", bufs=1, space="PSUM") as pp,
        ):
            # --- resident weights ---
            wc_sb = wpool.tile([128, KT, O], BF16)
            wg_sb = wpool.tile([128, KT, O], F8E4)
            wd_sb = wpool.tile([128, KT, R], F8E4)
            wu_sb = wpool.tile([R, O], BF16)
            bias_bc = wpool.tile([128, O], F32)

            # group-0 x tiles, hoisted so their DMAs issue early
            xb0_sb = xpool.tile([128, KT, TG], BF16, tag="xb")
            x80_sb = xpool.tile([128, KT, TG], F8E4, tag="x8")

            # DMA issue order = PE consumption order
            nc.sync.dma_start(out=wd_sb[:, :, :], in_=wd8_v[:, :, :])
            nc.sync.dma_start(out=x80_sb[:, :, :], in_=x8_v[:, :, 0:TG])
            nc.sync.dma_start(out=xb0_sb[:, :, :], in_=xb_v[:, :, 0:TG])
            for kk in range(KT // 2):
                nc.sync.dma_start(
                    out=wc_sb[:, 2 * kk : 2 * kk + 2, :],
                    in_=wcb_v[:, 2 * kk : 2 * kk + 2, :],
                )
            nc.sync.dma_start(out=wu_sb[:, :], in_=wub[:, :])
            nc.sync.dma_start(out=bias_bc[:, 0:512], in_=biasbc[:, 0:512])
            nc.sync.dma_start(out=bias_bc[:, 512:1024], in_=biasbc[:, 512:1024])
            nc.sync.dma_start(out=wg_sb[:, 0:4, :], in_=wg8_v[:, 0:4, :])
            nc.sync.dma_start(out=wg_sb[:, 4:8, :], in_=wg8_v[:, 4:8, :])

            # HAM spin-up: a short junk spin covers engine startup until the
            # first DMAs land; real matmuls keep the PE busy from then on
            junk = wpool.tile([128, 512], BF16)
            nc.vector.memset(junk[:, :], 0.0)
            warm = pp.tile([128, 512], F32, tag="warm")
            spin = None
            for i in range(6):
                spin = nc.tensor.matmul(warm[:, :], junk[:, 0:128], junk[:, :],
                                        start=True, stop=True)

            # warm-up matmuls observing the weight-DMA semaphores that real
            # matmul chains could not take as their single hw sync-wait
            warm_wd = nc.tensor.matmul(warm[0:16, 0:16], wd_sb[:, 0, :],
                                       wd_sb[:, 0, :], start=True, stop=True)
            warm_wc = nc.tensor.matmul(warm[0:1, 0:128], wc_sb[:, 0, 0:1],
                                       wc_sb[:, 0, 0:128], start=True,
                                       stop=True)
            warms = [warm_wd, warm_wc, spin]
            first_real = []

            for g in range(NG):
                tg0 = g * TG
                if g == 0:
                    xb_sb, x8_sb = xb0_sb, x80_sb
                else:
                    xb_sb = xpool.tile([128, KT, TG], BF16, tag="xb")
                    nc.sync.dma_start(
                        out=xb_sb[:, :, :], in_=xb_v[:, :, tg0 : tg0 + TG]
                    )
                    x8_sb = xpool.tile([128, KT, TG], F8E4, tag="x8")
                    nc.sync.dma_start(
                        out=x8_sb[:, :, :], in_=x8_v[:, :, tg0 : tg0 + TG]
                    )

                # LoRA down-projection for the whole 512-token group: [R, TG]
                # (values are 64*down because of the wd8 pre-scale)
                dps = pp.tile([R, TG], F32, tag="misc")
                for kk in range(KT // 2):
                    mm = nc.tensor.matmul(
                        dps[:, :],
                        wd_sb[:, 2 * kk : 2 * kk + 2, :],
                        x8_sb[:, 2 * kk : 2 * kk + 2, :],
                        start=(kk == 0),
                        stop=(kk == KT // 2 - 1),
                        perf_mode=DR,
                    )
                    if g == 0 and kk == 0:
                        first_real.append(mm)
                down_sb = epool.tile([R, TG], BF16, tag="down")
                nc.vector.tensor_copy(down_sb[:, :], dps[:, :])

                for t in range(TG // 128):
                    tsl = slice(t * 128, (t + 1) * 128)
                    out_sb = opool.tile([128, O], F32, tag="out")
                    osl = [slice(oh * 512, (oh + 1) * 512) for oh in range(NH)]
                    mps = [pp.tile([128, 512], F32, tag=f"main{oh}",
                                    name=f"mps{oh}")
                           for oh in range(NH)]
                    gps = [pp.tile([128, 512], F32, tag=f"gate{oh}",
                                    name=f"gps{oh}")
                           for oh in range(NH)]
                    lps = [pp.tile([128, 512], F32, tag=f"lora{oh}",
                                    name=f"lps{oh}")
                           for oh in range(NH)]
                    for k in range(KT):
                        for oh in range(NH):
                            mm = nc.tensor.matmul(
                                mps[oh][:, :],
                                xb_sb[:, k, tsl],
                                wc_sb[:, k, osl[oh]],
                                start=(k == 0),
                                stop=(k == KT - 1),
                            )
                            if g == 0 and t == 0 and k == 0 and oh == 0:
                                first_real.append(mm)
                    for kk in range(KT // 2):
                        for oh in range(NH):
                            nc.tensor.matmul(
                                gps[oh][:, :],
                                x8_sb[:, 2 * kk : 2 * kk + 2, tsl],
                                wg_sb[:, 2 * kk : 2 * kk + 2, osl[oh]],
                                start=(kk == 0),
                                stop=(kk == KT // 2 - 1),
                                perf_mode=DR,
                            )
                    for oh in range(NH):
                        nc.tensor.matmul(
                            lps[oh][:, :],
                            down_sb[:, tsl],
                            wu_sb[:, osl[oh]],
                            start=True,
                            stop=True,
                        )
                    for oh in range(NH):
                        # epilogue: VectorE drains mps early (GpSimd cannot
                        # read PSUM) while ScalarE computes the sigmoid
                        # (whose scale undoes the fp8 x64 weight pre-scale);
                        # the final SBUF-only add goes to GpSimd
                        m2_sb = epool.tile([128, 512], F32, tag=f"m2{oh}")
                        nc.vector.tensor_tensor(
                            m2_sb[:, :], mps[oh][:, :], bias_bc[:, osl[oh]],
                            add
                        )
                        g_sb = epool.tile([128, 512], F32, tag=f"sig{oh}")
                        nc.scalar.activation(
                            g_sb[:, :], gps[oh][:, :], sig, scale=1.0 / WS
                        )
                        gl_sb = epool.tile([128, 512], F32, tag=f"gl{oh}")
                        nc.vector.tensor_tensor(
                            gl_sb[:, :], g_sb[:, :], lps[oh][:, :], mult
                        )
                        nc.gpsimd.tensor_tensor(
                            out_sb[:, osl[oh]], gl_sb[:, :], m2_sb[:, :], add
                        )
                    nc.sync.dma_start(
                        out=out[tg0 + t * 128 : tg0 + (t + 1) * 128, :],
                        in_=out_sb[:, :],
                    )

            # ordering-only deps: the warm-ups precede the first matmul of
            # the group-0 chains, so those chains' single hw sync-wait slot
            # is free for their x-tile DMA semaphore
            for w in warms:
                for fr in first_real:
                    add_dep_helper(fr.ins, w.ins, False,
                                   "warmups before real matmuls")
    nc.compile()
    return nc


_NC_CACHE = None


def _get_nc():
    global _NC_CACHE
    if _NC_CACHE is None:
        _NC_CACHE = _build_nc()
    return _NC_CACHE


def _prep_inputs(x, W, b, W_down, W_up, W_gate, W_res):
    x = np.asarray(x, dtype=np.float32).reshape(TOK, I)
    wcb = np.ascontiguousarray(
        (np.asarray(W) + np.asarray(W_res)).T.astype(_BF16)
    )
    wg8 = np.ascontiguousarray((WS * np.asarray(W_gate)).T.astype(_F8))
    wd8 = np.ascontiguousarray((WS * np.asarray(W_down)).T.astype(_F8))
    wub = np.ascontiguousarray(((2.0 / WS) * np.asarray(W_up)).T.astype(_BF16))
    biasbc = np.ascontiguousarray(
        np.broadcast_to(np.asarray(b, dtype=np.float32).reshape(1, O), (128, O))
    )
    in_maps = []
    for c in range(N_CORES):
        xt_c = np.ascontiguousarray(x[c * T : (c + 1) * T, :].T)
        in_maps.append(
            {
                "xb": xt_c.astype(_BF16),
                "x8": xt_c.astype(_F8),
                "wcb": wcb,
                "wg8": wg8,
                "wd8": wd8,
                "wub": wub,
                "biasbc": biasbc,
            }
        )
    return in_maps


def run(inputs, trace=False, **kwargs):
    """Build + run on the 8 NeuronCores. Returns (full_output, BassKernelResults)."""
    nc = _get_nc()
    in_maps = _prep_inputs(**inputs)
    res = run_bass_kernel_spmd(
        nc, in_maps, list(range(N_CORES)), trace=trace, **kwargs
    )
    shards = [res.results[c]["out"] for c in range(N_CORES)]
    full = np.concatenate(shards, axis=0).reshape(B, S, O)
    return full, res


def kernel(**inputs):
    out, _ = run(inputs, trace=False)
    return out


# revision 7
# speedup vs baseline: 1.5135x; 1.0355x over previous
"""EnhancedLoRALinear Trainium2 kernel.

Computes, for x:[4,8192,1024] and torch-style weights (out,in):
    out = x @ (W + W_res)^T + b + sigmoid(x @ W_gate^T) * (2 * (x @ W_down^T) @ W_up^T)

Strategy (v4):
  - Data-parallel: the 32768 tokens are split across 8 NeuronCores (4096 each);
    the small weight matrices are replicated.
  - Algebraic fold: main + residual share one matmul with Wc = W + W_res.
  - Dtypes by accuracy need (tolerance 2e-2, measured ~4e-3):
      main path  : bf16 x / bf16 Wc           (full-rate MM, FWL on LDWEIGHTS)
      gate path  : fp8e4 x / fp8e4 64*W_gate  (DoubleRow: 2 k-tiles per MM,
                   the x64 weight scale keeps values out of fp8 subnormals;
                   undone for free via sigmoid's scale=1/64)
      down path  : fp8e4 DoubleRow with a widened W_down that also emits a
                   copy of the down-projection on partitions 32:48, so the
                   up-projection for both output halves runs as two
                   concurrent row-tiled (tile_position) matmuls
      up path    : bf16 with (2/64)*W_up
  - All tensors are host-swizzled into SBUF layout so every DMA moves
    per-partition-contiguous bytes (full HBM bandwidth, few descriptors).
  - Prologue: DMAs are issued in PE consumption order (wd2, x8/xb chunks of
    group 0 interleaved with Wc k-pairs, bias/W_up early, W_gate halves) so
    real matmuls start as soon as the first operands land. A short junk spin
    covers engine startup for the HAM clock gate.
  - Matmuls per 128-token tile are interleaved over the two 512-wide output
    halves (k-outer), giving stationary loads a two-matmul hiding window.
  - Epilogue: VectorE drains mps early (m2 = mps + bias; GpSimd cannot read
    PSUM), ScalarE does the sigmoid, VectorE the gate*lora, GpSimd the final
    SBUF-only add (VectorE for the last tile to shorten the tail).
"""

import ml_dtypes
import numpy as np

_BF16 = ml_dtypes.bfloat16
_F8 = ml_dtypes.float8_e4m3

import concourse.bass as bass
import concourse.bacc as bacc
import concourse.mybir as mybir
import concourse.tile as tile
from concourse.bass_utils import run_bass_kernel_spmd
from concourse.tile_rust import add_dep_helper

N_CORES = 8
B, S = 4, 8192
TOK = B * S                  # 32768 tokens total
T = TOK // N_CORES           # 4096 tokens per core
I = 1024                     # in_features
O = 1024                     # out_features
R = 16                       # lora rank
KT = I // 128                # 8 contraction tiles
TG = 512                     # token group (down-projection batch)
NG = T // TG                 # 8 groups per core
NH = O // 512                # 2 output halves
WS = 64.0                    # fp8 weight pre-scale (power of two, exact)
RW = 48                      # widened down-projection rows (16 + 16 zero + 16)

F32 = mybir.dt.float32
BF16 = mybir.dt.bfloat16
F8E4 = mybir.dt.float8e4
DR = mybir.MatmulPerfMode.DoubleRow


def _build_nc():
    nc = bacc.Bacc(None)

    # all inputs pre-swizzled to SBUF layout (partition-contiguous lines)
    xb = nc.dram_tensor("xb", [128, NG * KT, TG], BF16, kind="ExternalInput")
    x8 = nc.dram_tensor("x8", [128, NG * KT, TG], F8E4, kind="ExternalInput")
    wcb = nc.dram_tensor("wcb", [128, KT, O], BF16, kind="ExternalInput")
    wg8 = nc.dram_tensor("wg8", [128, KT, O], F8E4, kind="ExternalInput")
    wd2 = nc.dram_tensor("wd2", [128, KT, RW], F8E4, kind="ExternalInput")
    wu2 = nc.dram_tensor("wu2", [64, 512], BF16, kind="ExternalInput")
    biasbc = nc.dram_tensor("biasbc", [128, O], F32, kind="ExternalInput")
    out = nc.dram_tensor("out", [T, O], F32, kind="ExternalOutput")

    sig = mybir.ActivationFunctionType.Sigmoid
    mult = mybir.AluOpType.mult
    add = mybir.AluOpType.add

    with tile.TileContext(nc) as tc:
        with (
            tc.tile_pool(name="wpool", bufs=1) as wpool,
            tc.tile_pool(name="xpool", bufs=3) as xpool,
            tc.tile_pool(name="opool", bufs=3) as opool,
            tc.tile_pool(name="epool", bufs=3) as epool,
            tc.tile_pool(name="psum", bufs=1, space="PSUM") as pp,
        ):
            # --- resident weights ---
            wc_sb = wpool.tile([128, KT, O], BF16)
            wg_sb = wpool.tile([128, KT, O], F8E4)
            wd_sb = wpool.tile([128, KT, RW], F8E4)
            wu_sb = wpool.tile([64, 512], BF16)
            bias_bc = wpool.tile([128, O], F32)

            # group-0 x tiles, hoisted so their DMAs issue early
            xb0_sb = xpool.tile([128, KT, TG], BF16, tag="xb")
            x80_sb = xpool.tile([128, KT, TG], F8E4, tag="x8")

            # DMA issue order = PE consumption order (group 0 pipelined)
            nc.sync.dma_start(out=wd_sb[:, :, :], in_=wd2[:, :, :])
            nc.sync.dma_start(out=x80_sb[:, :, :], in_=x8[:, 0:KT, :])
            nc.sync.dma_start(out=xb0_sb[:, 0:2, :], in_=xb[:, 0:2, :])
            nc.sync.dma_start(out=wc_sb[:, 0:2, :], in_=wcb[:, 0:2, :])
            nc.sync.dma_start(out=bias_bc[:, :], in_=biasbc[:, :])
            nc.sync.dma_start(out=wu_sb[:, :], in_=wu2[:, :])
            nc.sync.dma_start(out=xb0_sb[:, 2:4, :], in_=xb[:, 2:4, :])
            nc.sync.dma_start(out=wc_sb[:, 2:4, :], in_=wcb[:, 2:4, :])
            nc.sync.dma_start(out=wg_sb[:, 0:4, :], in_=wg8[:, 0:4, :])
            nc.sync.dma_start(out=xb0_sb[:, 4:6, :], in_=xb[:, 4:6, :])
            nc.sync.dma_start(out=wc_sb[:, 4:6, :], in_=wcb[:, 4:6, :])
            nc.sync.dma_start(out=xb0_sb[:, 6:8, :], in_=xb[:, 6:8, :])
            nc.sync.dma_start(out=wc_sb[:, 6:8, :], in_=wcb[:, 6:8, :])
            nc.sync.dma_start(out=wg_sb[:, 4:8, :], in_=wg8[:, 4:8, :])

            # HAM spin-up: a short junk spin covers engine startup until the
            # first DMAs land; real matmuls keep the PE busy from then on
            junk = wpool.tile([128, 512], BF16)
            nc.vector.memset(junk[:, :], 0.0)
            warm = pp.tile([128, 512], F32, tag="warm")
            spin = None
            for i in range(10):
                spin = nc.tensor.matmul(warm[:, :], junk[:, 0:128], junk[:, :],
                                        start=True, stop=True)

            # warm-up matmuls observing the weight-DMA semaphores of the two
            # chains that would otherwise need two semaphores at once
            warm_wd = nc.tensor.matmul(warm[0:RW, 0:RW], wd_sb[:, 0, :],
                                       wd_sb[:, 0, :], start=True, stop=True)
            warm_wc = nc.tensor.matmul(warm[0:1, 0:128], wc_sb[:, 0, 0:1],
                                       wc_sb[:, 0, 0:128], start=True,
                                       stop=True)
            warms = [warm_wd, warm_wc, spin]
            first_real = []

            for g in range(NG):
                tg0 = g * TG
                if g == 0:
                    xb_sb, x8_sb = xb0_sb, x80_sb
                else:
                    xb_sb = xpool.tile([128, KT, TG], BF16, tag="xb")
                    nc.sync.dma_start(
                        out=xb_sb[:, :, :],
                        in_=xb[:, g * KT : (g + 1) * KT, :],
                    )
                    x8_sb = xpool.tile([128, KT, TG], F8E4, tag="x8")
                    nc.sync.dma_start(
                        out=x8_sb[:, :, :],
                        in_=x8[:, g * KT : (g + 1) * KT, :],
                    )

                # LoRA down-projection for the whole 512-token group,
                # [RW, TG]: rows 0:16 = 64*down, 16:32 zero, 32:48 = 64*down
                dps = pp.tile([RW, TG], F32, tag="misc")
                for kk in range(KT // 2):
                    mm = nc.tensor.matmul(
                        dps[:, :],
                        wd_sb[:, 2 * kk : 2 * kk + 2, :],
                        x8_sb[:, 2 * kk : 2 * kk + 2, :],
                        start=(kk == 0),
                        stop=(kk == KT // 2 - 1),
                        perf_mode=DR,
                    )
                    if g == 0 and kk == 0:
                        first_real.append(mm)
                down_sb = epool.tile([RW, TG], BF16, tag="down")
                nc.vector.tensor_copy(down_sb[:, :], dps[:, :])

                for t in range(TG // 128):
                    tsl = slice(t * 128, (t + 1) * 128)
                    last_tile = g == NG - 1 and t == TG // 128 - 1
                    out_sb = opool.tile([128, O], F32, tag="out")
                    osl = [slice(oh * 512, (oh + 1) * 512) for oh in range(NH)]
                    mps = [pp.tile([128, 512], F32, tag=f"main{oh}",
                                   name=f"mps{oh}") for oh in range(NH)]
                    gps = [pp.tile([128, 512], F32, tag=f"gate{oh}",
                                   name=f"gps{oh}") for oh in range(NH)]
                    lps = [pp.tile([128, 512], F32, tag=f"lora{oh}",
                                   name=f"lps{oh}") for oh in range(NH)]
                    for k in range(KT):
                        for oh in range(NH):
                            mm = nc.tensor.matmul(
                                mps[oh][:, :],
                                xb_sb[:, k, tsl],
                                wc_sb[:, k, osl[oh]],
                                start=(k == 0),
                                stop=(k == KT - 1),
                            )
                            if g == 0 and t == 0 and k == 0 and oh == 0:
                                first_real.append(mm)
                    for kk in range(KT // 2):
                        for oh in range(NH):
                            nc.tensor.matmul(
                                gps[oh][:, :],
                                x8_sb[:, 2 * kk : 2 * kk + 2, tsl],
                                wg_sb[:, 2 * kk : 2 * kk + 2, osl[oh]],
                                start=(kk == 0),
                                stop=(kk == KT // 2 - 1),
                                perf_mode=DR,
                            )
                    # up-projection: two concurrent row-tiled matmuls (the
                    # stationary/moving APs at base partition 32*oh derive
                    # tile_position row groups 0 and 1)
                    for oh in range(NH):
                        nc.tensor.matmul(
                            lps[oh][:, :],
                            down_sb[32 * oh : 32 * oh + R, tsl],
                            wu_sb[32 * oh : 32 * oh + R, :],
                            start=True,
                            stop=True,
                        )
                    for oh in range(NH):
                        # epilogue: VectorE drains mps early (GpSimd cannot
                        # read PSUM) while ScalarE computes the sigmoid
                        # (whose scale undoes the fp8 x64 weight pre-scale);
                        # the final SBUF-only add goes to GpSimd
                        m2_sb = epool.tile([128, 512], F32, tag=f"m2{oh}")
                        nc.vector.tensor_tensor(
                            m2_sb[:, :], mps[oh][:, :], bias_bc[:, osl[oh]],
                            add
                        )
                        g_sb = epool.tile([128, 512], F32, tag=f"sig{oh}")
                        nc.scalar.activation(
                            g_sb[:, :], gps[oh][:, :], sig, scale=1.0 / WS
                        )
                        gl_sb = epool.tile([128, 512], F32, tag=f"gl{oh}")
                        nc.vector.tensor_tensor(
                            gl_sb[:, :], g_sb[:, :], lps[oh][:, :], mult
                        )
                        adder = nc.vector if last_tile else nc.gpsimd
                        adder.tensor_tensor(
                            out_sb[:, osl[oh]], gl_sb[:, :], m2_sb[:, :], add
                        )
                        if last_tile:
                            nc.sync.dma_start(
                                out=out[tg0 + t * 128 : tg0 + (t + 1) * 128,
                                        osl[oh]],
                                in_=out_sb[:, osl[oh]],
                            )
                    if not last_tile:
                        nc.sync.dma_start(
                            out=out[tg0 + t * 128 : tg0 + (t + 1) * 128, :],
                            in_=out_sb[:, :],
                        )

            # ordering-only deps: the warm-ups precede the first matmul of
            # the group-0 chains, so those chains' hw sync-wait slot is free
            # for their x-tile DMA semaphore
            for w in warms:
                for fr in first_real:
                    add_dep_helper(fr.ins, w.ins, False,
                                   "warmups before real matmuls")
    nc.compile()
    return nc


_NC_CACHE = None


def _get_nc():
    global _NC_CACHE
    if _NC_CACHE is None:
        _NC_CACHE = _build_nc()
    return _NC_CACHE


def _swz(a, free):
    """[I, F] -> [128, KT, F] partition-contiguous swizzle."""
    return np.ascontiguousarray(a.reshape(KT, 128, free).transpose(1, 0, 2))


def _prep_inputs(x, W, b, W_down, W_up, W_gate, W_res):
    x = np.asarray(x, dtype=np.float32).reshape(TOK, I)
    wcb = _swz((np.asarray(W) + np.asarray(W_res)).T.astype(_BF16), O)
    wg8 = _swz((WS * np.asarray(W_gate)).T.astype(_F8), O)
    # widened down weights: columns 0:16 = 64*Wd^T, 16:32 = 0, 32:48 = 64*Wd^T
    wdt = (WS * np.asarray(W_down)).T.astype(_F8)          # [I, R]
    wd2 = np.zeros((I, RW), dtype=_F8)
    wd2[:, 0:R] = wdt
    wd2[:, 2 * R : 3 * R] = wdt
    wd2 = _swz(wd2, RW)
    # packed up weights: rows 0:16 -> half 0, rows 32:48 -> half 1
    wut = ((2.0 / WS) * np.asarray(W_up)).T.astype(_BF16)  # [R, O]
    wu2 = np.zeros((64, 512), dtype=_BF16)
    wu2[0:R, :] = wut[:, 0:512]
    wu2[2 * R : 3 * R, :] = wut[:, 512:1024]
    biasbc = np.ascontiguousarray(
        np.broadcast_to(np.asarray(b, dtype=np.float32).reshape(1, O), (128, O))
    )
    in_maps = []
    for c in range(N_CORES):
        xt_c = np.ascontiguousarray(x[c * T : (c + 1) * T, :].T)  # [I, T]
        # [I, T] -> [128, NG*KT, TG]: per-group-per-ktile contiguous lines
        xs = xt_c.reshape(KT, 128, NG, TG).transpose(1, 2, 0, 3)
        xs = np.ascontiguousarray(xs).reshape(128, NG * KT, TG)
        in_maps.append(
            {
                "xb": xs.astype(_BF16),
                "x8": xs.astype(_F8),
                "wcb": wcb,
                "wg8": wg8,
                "wd2": wd2,
                "wu2": wu2,
                "biasbc": biasbc,
            }
        )
    return in_maps


def run(inputs, trace=False, **kwargs):
    """Build + run on the 8 NeuronCores. Returns (full_output, BassKernelResults)."""
    nc = _get_nc()
    in_maps = _prep_inputs(**inputs)
    res = run_bass_kernel_spmd(
        nc, in_maps, list(range(N_CORES)), trace=trace, **kwargs
    )
    shards = [res.results[c]["out"] for c in range(N_CORES)]
    full = np.concatenate(shards, axis=0).reshape(B, S, O)
    return full, res


def kernel(**inputs):
    out, _ = run(inputs, trace=False)
    return out
